# revision 25
# baseline (speedup 1.0000x reference)
# Trainium2 Bass kernel for nn_Edge_CNN (DynamicEdgeConv x3 + PairNorm + pool + MLP head).
#
# Strategy: data-parallel over the 32 graphs -> 8 NeuronCores x 4 graphs. PairNorm
# couples all graphs (stats over the whole batch), handled with a tiny per-layer
# AllReduce of per-channel sums + total square sum.
#
# Key structure:
#  - PairNorm is a monotone per-channel affine (scalar scale s>0, per-channel
#    shift). kNN ordering is affine-invariant, and u+max_j v commutes with the
#    affine, so each layer computes on UNNORMALIZED fp32 features z and the
#    normalization collapses into the final relu: z_next = relu(s*m + ccol),
#    ccol = b - s*W[:D]^T mu. All collective-consuming ops are emitted AFTER the
#    next layer's selection work, so the per-layer AllReduce latency hides
#    behind compute (engines execute their streams in order).
#  - per layer, emission is phased (sel x4 graphs, agg x4, consume, relu x4) so
#    the in-order engine streams pipeline across graphs and layers.
#  - top-k (k=10, self-inclusive) per row: gram chunk in PSUM (fp32 matmuls,
#    -|x_j|^2/2 as a rank-1 accumulate), ACT-copied to SBUF, then
#    max8/max_index/match_replace/max8/max_index; slots 0..9 of the 16-slot
#    gather layout are ranks 1..10. Index transposes run on the PE (u16 idx ->
#    fp32 cast -> transpose-matmul vs identity -> ACT cast back).
#  - aggregation: gpsimd ap_gather of v columns (fp32; L3 packs its two 128-ch
#    blocks as fp16 pairs in one u32 gather); segmented max over slots 0..9 via
#    DVE strided tensor_reduce (L1/L2, fp16 out) or packed-fp16 TT trees (L3,
#    2x mode); maxv is added into the u PSUM accumulation with an fp16
#    identity matmul (L1/L2) and relu reads m fp16.
#
# kernel(**inputs) takes the FULL unsharded inputs and returns the FULL [32, 2].

import numpy as np
from contextlib import ExitStack

import concourse.bass as bass
import concourse.bacc as bacc
import concourse.mybir as mybir
import concourse.tile as tile
from concourse.bass_utils import run_bass_kernel_spmd

N = 1024
B_TOTAL = 32
N_CORES = 8
G = B_TOTAL // N_CORES
F32 = mybir.dt.float32
F16 = mybir.dt.float16
U16 = mybir.dt.uint16
I16 = mybir.dt.int16
AF = mybir.ActivationFunctionType
ALU = mybir.AluOpType
AX = mybir.AxisListType
NCHUNK = N // 128
LAYERS = [(2, 64), (64, 128), (128, 256)]  # (D_in, C_out)
HN = 128  # nodes per gather part


def _ccdiv(c):
    return (c + 127) // 128


def _build(tc, nc, ins, outs, n_cores, eps=1e-5):
    TOTAL_NODES = float(B_TOTAL * N)
    replica = [list(range(n_cores))]

    es = ExitStack()
    sb = es.enter_context(tc.tile_pool(name="sb", bufs=1))
    xp = es.enter_context(tc.tile_pool(name="xp", bufs=12))
    wk = es.enter_context(tc.tile_pool(name="wk", bufs=2))
    tcp = es.enter_context(tc.tile_pool(name="tcp", bufs=3))
    mp = es.enter_context(tc.tile_pool(name="mp", bufs=9))
    vt = es.enter_context(tc.tile_pool(name="vt", bufs=5))
    sm = es.enter_context(tc.tile_pool(name="sm", bufs=4))
    gt = es.enter_context(tc.tile_pool(name="gt", bufs=2))
    ndp = es.enter_context(tc.tile_pool(name="ndp", bufs=2))
    ps_g = es.enter_context(tc.tile_pool(name="ps_g", bufs=3, space="PSUM"))
    ps_t = es.enter_context(tc.tile_pool(name="ps_t", bufs=1, space="PSUM"))
    ps_v = es.enter_context(tc.tile_pool(name="ps_v", bufs=2, space="PSUM"))
    dr = es.enter_context(tc.tile_pool(name="dr", bufs=1, space="DRAM"))

    onesD = sb.tile([128, 1], F32, tag="onesD")
    nc.vector.memset(onesD[:], 1.0)
    ones1 = sb.tile([1, 128], F32, tag="ones1")
    nc.vector.memset(ones1[:], 1.0)
    ones128 = sb.tile([128, 1], F32, tag="ones128")
    nc.vector.memset(ones128[:], 1.0)
    rcol0 = sb.tile([128, 1], F32, tag="rcol0")
    nc.vector.memset(rcol0[:], 1.0)
    # z (layer-input feature) tiles (loaded first so layer 0 starts early)
    zT = {}
    for g in range(G):
        t = xp.tile([128, N], F32, tag="xh", name=f"x0_{g}")
        nc.sync.dma_start(t[0:2, :], ins["pos"][g, :, :])
        zT[(g, 0)] = t
    identF = sb.tile([128, 128], F32, tag="identF")
    nc.sync.dma_start(identF[:], ins["ident"][:])
    identH = sb.tile([128, 128], F16, tag="identH")
    nc.sync.dma_start(identH[:], ins["identh"][:])

    W = {}
    for li, (D, C) in enumerate(LAYERS):
        for nm in ("wab", "wb"):
            t = sb.tile([D, C], F32, tag=f"{nm}{li}", name=f"{nm}{li}")
            nc.sync.dma_start(t[:], ins[f"{nm}{li}"][:])
            W[f"{nm}{li}"] = t
        if li > 0:
            t = sb.tile([D, C], F32, tag=f"wa{li}", name=f"wa{li}")
            nc.sync.dma_start(t[:], ins[f"wa{li}"][:])
            W[f"wa{li}"] = t
        t = sb.tile([min(C, 128), _ccdiv(C)], F32, tag=f"b{li}", name=f"b{li}")
        nc.sync.dma_start(t[:], ins[f"b{li}"][:].rearrange("(cc p) one -> p (cc one)",
                                                           p=min(C, 128)))
        W[f"b{li}"] = t
    wl1 = sb.tile([128, 2, 64], F32, tag="wl1")
    nc.sync.dma_start(wl1[:], ins["wl1"][:].rearrange("cc p c -> p cc c"))
    wl2 = sb.tile([64, 2], F32, tag="wl2")
    nc.sync.dma_start(wl2[:], ins["wl2"][:])
    bl1 = sb.tile([64, 1], F32, tag="bl1")
    nc.sync.dma_start(bl1[:], ins["bl1"][:])
    bl2 = sb.tile([2, 1], F32, tag="bl2")
    nc.sync.dma_start(bl2[:], ins["bl2"][:])

    # per-layer normalization constants (written after each layer's collective)
    rcolL = {-1: rcol0}
    ccol = {}  # (li, cc) -> [128,1] f32 bias column for the relu of layer li
    for cc in range(_ccdiv(LAYERS[0][1])):
        ccol[(0, cc)] = None  # layer 0 uses b0 directly

    def unit_sel(li, D, C, g):
        CC = _ccdiv(C)
        zin = zT[(g, 0)]

        # ---- selection prep: rhsq = -0.5*sum_c z^2 (fp16 row)
        sqx = sm.tile([128, N], F32, tag="sqx")
        nc.scalar.activation(sqx[0:D, :], zin[0:D, :], AF.Square)
        psq = ps_v.tile([1, N], F32, tag="pvu")
        for b in range(2):
            sl = slice(512 * b, 512 * (b + 1))
            nc.tensor.matmul(psq[:, sl], onesD[0:D, :], sqx[0:D, sl],
                             start=True, stop=True)
        rhsq = sm.tile([1, N], F32, tag="rhsq")
        nc.scalar.activation(rhsq[:], psq[:], AF.Copy, scale=-0.5)

        # ---- per 128-row chunk: gram into PSUM, copy to SBUF (ACT), exact
        # top-16 selection on the SBUF copy; index transposes on the PE.
        wrapIdx = sm.tile([16, N], U16, tag="wrapIdx")
        for c in range(NCHUNK):
            nd = ndp.tile([128, N], F32, tag="nd")
            for b in range(2):
                sl = slice(512 * b, 512 * (b + 1))
                pg = ps_g.tile([128, 512], F32, tag="gram")
                nc.tensor.matmul(pg[:], zin[0:D, 128 * c:128 * (c + 1)],
                                 zin[0:D, sl], start=True, stop=False)
                nc.tensor.matmul(pg[:], ones1[:], rhsq[:, sl],
                                 start=False, stop=True)
                nc.scalar.activation(nd[:, sl], pg[:], AF.Copy)
            mx = sm.tile([128, 16], F32, tag="mx")
            half = (c % 2) * 16
            if half == 0:
                idx32 = sm.tile([128, 32], U16, tag="idx32")
            nc.vector.max(mx[:, 0:8], nd[:])
            nc.vector.max_index(idx32[:, half:half + 8], mx[:, 0:8], nd[:])
            nc.vector.match_replace(nd[:], mx[:, 0:8], nd[:], -1e30)
            nc.vector.max(mx[:, 8:16], nd[:])
            nc.vector.max_index(idx32[:, half + 8:half + 16], mx[:, 8:16], nd[:])
            if half == 16:
                idx32f = sm.tile([128, 32], F32, tag="idx32f")
                nc.scalar.activation(idx32f[:], idx32[:], AF.Copy)
                for h2 in range(2):
                    ptr = ps_t.tile([16, 128], F32, tag="ptr")
                    nc.tensor.transpose(ptr[:], idx32f[:, 16 * h2:16 * (h2 + 1)],
                                        identF[:])
                    nc.scalar.activation(
                        wrapIdx[0:16, 128 * (c - 1 + h2):128 * (c + h2)],
                        ptr[:], AF.Copy)

        repIdx = sm.tile([128, N], I16, tag="repIdx")
        for grp in range(8):
            nc.sync.dma_start(repIdx[16 * grp:16 * (grp + 1), :],
                              wrapIdx[0:16, :].bitcast(I16))

        # ---- v transforms (raw, unnormalized)
        if li < 2:
            vtt = vt.tile([128, N], F32, tag="vT")
            pv = ps_v.tile([128, N], F32, tag="pvu")
            for b in range(2):
                sl = slice(512 * b, 512 * (b + 1))
                nc.tensor.matmul(pv[0:C, sl], W[f"wb{li}"][0:D, :], zin[0:D, sl],
                                 start=True, stop=True)
            nc.scalar.activation(vtt[0:C, :], pv[0:C, :], AF.Copy)
        else:
            # pack the two 128-ch blocks as fp16 pairs into one fp32-typed tile
            vtt = vt.tile([128, N], F32, tag="vT")
            v16 = vtt[:].bitcast(F16).rearrange("p (n two) -> p n two", two=2)
            for cc in range(CC):
                csl = slice(128 * cc, 128 * (cc + 1))
                pv = ps_v.tile([128, N], F32, tag="pvu")
                for b in range(2):
                    sl = slice(512 * b, 512 * (b + 1))
                    nc.tensor.matmul(pv[:, sl], W[f"wb{li}"][0:D, csl], zin[0:D, sl],
                                     start=True, stop=True)
                nc.scalar.activation(v16[:, :, cc:cc + 1], pv[:], AF.Copy)

        return (repIdx, vtt)

    def unit_agg(li, D, C, g, sel):
        CC = _ccdiv(C)
        repIdx, vtt = sel
        zin = zT[(g, 0)]
        ch = min(C, 128)

        # ---- gather 16 slots per node (slots 0..9 = ranks 1..10), then
        # segmented max over slots 0..9 on DVE: strided tensor_reduce for
        # L1/L2 (fp32 -> fp16 maxv), packed-fp16-pair TT tree for L3 (2x mode).
        tC = tcp.tile([128, N], F32, tag="tC")
        tC16 = tC[:].bitcast(F16)
        for part in range(N // HN):
            hsl = slice(part * HN, (part + 1) * HN)
            gout = gt.tile([128, 16 * HN], F32, tag="gout", name=f"gout{part}")
            nc.gpsimd.ap_gather(gout[0:ch, :], vtt[0:ch, :], repIdx[0:ch, hsl],
                                channels=ch, num_elems=N, d=1, num_idxs=16 * HN)
            if li < 2:
                gv = gout[0:ch, :].rearrange("p (n s) -> p n s", s=16)
                nc.vector.tensor_reduce(tC16[0:ch, hsl], gv[:, :, 0:10],
                                        AX.X, ALU.max)
            else:
                gv = gout[:].bitcast(F16).rearrange("p (n s t) -> p n s t", s=16, t=2)
                tA = wk.tile([128, 5 * HN], F32, tag="tA")
                tAv = tA[:].bitcast(F16).rearrange("p (n s t) -> p n s t", s=5, t=2)
                nc.vector.tensor_tensor(tAv, gv[:, :, 0:5, :], gv[:, :, 5:10, :],
                                        ALU.max)
                tB = wk.tile([128, 2 * HN], F32, tag="tB")
                tBv = tB[:].bitcast(F16).rearrange("p (n s t) -> p n s t", s=2, t=2)
                nc.vector.tensor_tensor(tBv, tAv[:, :, 0:2, :], tAv[:, :, 2:4, :],
                                        ALU.max)
                tCv = tC16[:, 2 * part * HN:2 * (part + 1) * HN].rearrange(
                    "p (n one t) -> p n one t", one=1, t=2)
                nc.vector.tensor_tensor(tCv, tBv[:, :, 0:1, :], tBv[:, :, 1:2, :],
                                        ALU.max)
                nc.vector.tensor_tensor(tCv, tCv, tAv[:, :, 4:5, :], ALU.max)

        # ---- m = u + maxv (relu deferred to phase R). L1/L2: maxv folded into
        # the u PSUM accumulation via an fp16 identity matmul; L3: DVE adds the
        # strided fp16 maxv lane to the u PSUM directly.
        ms = {}
        for cc in range(CC):
            cw = min(128, C - 128 * cc)
            csl = slice(128 * cc, 128 * cc + cw)
            pu = ps_v.tile([128, N], F32, tag="pvu")
            for b in range(2):
                sl = slice(512 * b, 512 * (b + 1))
                nc.tensor.matmul(pu[0:cw, sl], W[f"wab{li}"][0:D, csl], zin[0:D, sl],
                                 start=True, stop=False)
                if li < 2:
                    nc.tensor.matmul(pu[0:cw, sl], identH[0:cw, 0:cw], tC16[0:cw, sl],
                                     start=False, stop=True)
                else:
                    nc.tensor.matmul(
                        pu[0:cw, sl], identH[0:cw, 0:cw],
                        tC16[:].rearrange("p (n t) -> p n t", t=2)[:, sl, cc:cc + 1],
                        start=False, stop=True)
            m = mp.tile([128, N], F16, tag="mT", name=f"m{li}_{g}_{cc}")
            nc.scalar.activation(m[0:cw, :], pu[0:cw, :], AF.Copy)
            ms[cc] = (m, cw)
        return ms

    def unit_relu(li, C, g, ms):
        hs = {}
        for cc, (m, cw) in ms.items():
            z = xp.tile([128, N], F32, tag="xh", name=f"h{li}_{g}_{cc}")
            rsum = sm.tile([128, 1], F32, tag="rsum")
            bias_col = W[f"b{li}"][0:cw, cc:cc + 1] if ccol[(li, cc)] is None \
                else ccol[(li, cc)][0:cw, :]
            nc.scalar.activation(z[0:cw, :], m[0:cw, :], AF.Relu,
                                 bias=bias_col, scale=rcolL[li - 1][0:cw, :],
                                 accum_out=rsum[0:cw, :])
            sqz = sm.tile([128, N], F16, tag="sqz")
            qsum = sm.tile([128, 1], F32, tag="qsum")
            nc.scalar.activation(sqz[0:cw, :], z[0:cw, :], AF.Square,
                                 accum_out=qsum[0:cw, :])
            hs[cc] = (z, rsum, qsum, cw)
        return hs

    def make_consume(li, C, CC, statg):
        def consume():
            # statg DMA back + all collective-consuming ops; emitted late so the
            # in-order engine streams never stall on the AllReduce latency.
            mu = sm.tile([128, CC], F32, tag=f"mu{li}", name=f"mu{li}")
            musq = sm.tile([128, CC], F32, tag=f"musq{li}", name=f"musq{li}")
            for cc in range(CC):
                nc.vector.tensor_scalar_mul(mu[:, cc:cc + 1],
                                            statg[:, 2 * cc:2 * cc + 1],
                                            1.0 / TOTAL_NODES)
            nc.vector.tensor_tensor(musq[:], mu[:], mu[:], ALU.mult)
            pr = ps_v.tile([1, 2], F32, tag="pvu")
            for cc in range(CC):
                nc.tensor.matmul(pr[:, 0:1], statg[:, 2 * cc + 1:2 * cc + 2],
                                 ones128[:], start=(cc == 0), stop=(cc == CC - 1))
            for cc in range(CC):
                nc.tensor.matmul(pr[:, 1:2], musq[:, cc:cc + 1], ones128[:],
                                 start=(cc == 0), stop=(cc == CC - 1))
            sc = sm.tile([1, 2], F32, tag=f"sc{li}", name=f"sc{li}")
            nc.scalar.activation(sc[:], pr[:], AF.Copy)
            rsc = sm.tile([1, 1], F32, tag=f"rsc{li}", name=f"rsc{li}")
            nc.vector.tensor_scalar(rsc[:], sc[:, 0:1], scalar1=1.0 / TOTAL_NODES,
                                    scalar2=eps, op0=ALU.mult, op1=ALU.add)
            nc.vector.tensor_tensor(rsc[:], rsc[:], sc[:, 1:2], ALU.subtract)
            nc.vector.reciprocal(rsc[:], rsc[:])
            nc.scalar.activation(rsc[:], rsc[:], AF.Sqrt)
            prb = ps_v.tile([128, 1], F32, tag="pvu")
            nc.tensor.matmul(prb[:], ones1[:], rsc[:], start=True, stop=True)
            rcol = sm.tile([128, 1], F32, tag=f"rcol{li}", name=f"rcol{li}")
            nc.scalar.activation(rcol[:], prb[:], AF.Copy)
            rcolL[li] = rcol
            if li < 2:
                nrcol = sm.tile([128, 1], F32, tag=f"nrcol{li}", name=f"nrcol{li}")
                nc.vector.tensor_scalar_mul(nrcol[:], rcol[:], -1.0)
                D2, C2 = LAYERS[li + 1]
                for cc2 in range(_ccdiv(C2)):
                    cw2 = min(128, C2 - 128 * cc2)
                    csl2 = slice(128 * cc2, 128 * cc2 + cw2)
                    pc = ps_v.tile([128, 1], F32, tag="pvu")
                    nc.tensor.matmul(pc[0:cw2, :], W[f"wa{li + 1}"][0:D2, csl2],
                                     mu[0:D2, 0:1], start=True, stop=True)
                    cl = sm.tile([128, 1], F32, tag=f"ccol{li + 1}_{cc2}",
                                 name=f"ccol{li + 1}_{cc2}")
                    nc.scalar.activation(cl[0:cw2, :], pc[0:cw2, :], AF.Identity,
                                         scale=nrcol[0:cw2, :],
                                         bias=W[f"b{li + 1}"][0:cw2, cc2:cc2 + 1])
                    ccol[(li + 1, cc2)] = cl
            else:
                muR = sm.tile([128, CC], F32, tag=f"muR{li}", name=f"muR{li}")
                nc.vector.tensor_scalar(muR[:], mu[:], scalar1=rcol[:], scalar2=None,
                                        op0=ALU.mult)
                ccol["muR3"] = muR
        return consume

    pending = None
    hsg = {}
    for li, (D, C) in enumerate(LAYERS):
        CC = _ccdiv(C)
        sels = {}
        for g in range(G):
            sels[g] = unit_sel(li, D, C, g)
        msg_ = {}
        for g in range(G):
            msg_[g] = unit_agg(li, D, C, g, sels[g])
        if pending is not None:
            pending()
        ssum = [sm.tile([128, G], F32, tag=f"ssum{li}_{cc}", name=f"ssum{li}_{cc}")
                for cc in range(CC)]
        qsum = [sm.tile([128, G], F32, tag=f"qsum{li}_{cc}", name=f"qsums{li}_{cc}")
                for cc in range(CC)]
        hsg = {}
        for g in range(G):
            hs = unit_relu(li, C, g, msg_[g])
            for cc, (h, rs, qs, cw) in hs.items():
                nc.vector.tensor_copy(ssum[cc][0:cw, g:g + 1], rs[0:cw, :])
                nc.vector.tensor_copy(qsum[cc][0:cw, g:g + 1], qs[0:cw, :])
                hsg[(g, cc)] = (h, cw)
        stat = sm.tile([128, 2 * CC], F32, tag=f"stat{li}", name=f"stat{li}")
        nc.vector.memset(stat[:], 0.0)
        for cc in range(CC):
            cw = min(128, C - 128 * cc)
            nc.vector.tensor_reduce(stat[0:cw, 2 * cc:2 * cc + 1], ssum[cc][0:cw, :],
                                    AX.X, ALU.add)
            nc.vector.tensor_reduce(stat[0:cw, 2 * cc + 1:2 * cc + 2], qsum[cc][0:cw, :],
                                    AX.X, ALU.add)
        bi = dr.tile([128, 2 * CC], F32, tag=f"cc_in{li}", name=f"cc_in{li}")
        bo = dr.tile([128, 2 * CC], F32, tag=f"cc_out{li}", name=f"cc_out{li}")
        nc.sync.dma_start(bi[:], stat[:])
        nc.gpsimd.collective_compute("AllReduce", ALU.add, replica_groups=replica,
                                     ins=[bi[:].opt()], outs=[bo[:].opt()])
        statg = sm.tile([128, 2 * CC], F32, tag=f"statg{li}", name=f"statg{li}")
        nc.sync.dma_start(statg[:], bo[:])
        pending = make_consume(li, C, CC, statg)
        if li < 2:
            for g in range(G):
                zT[(g, 0)] = hsg[(g, 0)][0]

    # ---- global max pool (on raw z3, before the last collective lands) +
    # affine + MLP head
    gmat = sm.tile([128, 2, G], F32, tag="gmat")
    for g in range(G):
        for cc in range(2):
            h, cw = hsg[(g, cc)]
            nc.vector.tensor_reduce(gmat[:, cc, g:g + 1], h[:], AX.X, ALU.max)
    pending()
    muR = ccol["muR3"]
    rcol3 = rcolL[2]
    for cc in range(2):
        nc.vector.tensor_scalar(gmat[:, cc, :], gmat[:, cc, :], scalar1=rcol3[:],
                                scalar2=muR[:, cc:cc + 1], op0=ALU.mult,
                                op1=ALU.subtract)
    ph = ps_v.tile([64, G], F32, tag="pvu")
    for cc in range(2):
        nc.tensor.matmul(ph[:], wl1[:, cc, :], gmat[:, cc, :], start=(cc == 0),
                         stop=(cc == 1))
    hh = sm.tile([64, G], F32, tag="hh")
    nc.scalar.activation(hh[:], ph[:], AF.Relu, bias=bl1[:])
    po = ps_v.tile([2, G], F32, tag="pvu")
    nc.tensor.matmul(po[:], wl2[:], hh[:], start=True, stop=True)
    oo = sm.tile([2, G], F32, tag="oo")
    nc.scalar.activation(oo[:], po[:], AF.Identity, bias=bl2[:])
    nc.sync.dma_start(outs["out"][:], oo[:])
    es.close()


def _host_weights(inputs):
    w = {}
    for li, (D, C) in enumerate(LAYERS):
        Wl = np.asarray(inputs[f"W{li + 1}"], dtype=np.float32)
        w[f"wab{li}"] = np.ascontiguousarray(Wl[:D] - Wl[D:])
        w[f"wb{li}"] = np.ascontiguousarray(Wl[D:])
        if li > 0:
            w[f"wa{li}"] = np.ascontiguousarray(Wl[:D])
        w[f"b{li}"] = np.ascontiguousarray(
            np.asarray(inputs[f"b{li + 1}"], dtype=np.float32).reshape(C, 1))
    w["wl1"] = np.ascontiguousarray(
        np.asarray(inputs["Wl1"], dtype=np.float32).reshape(2, 128, 64))
    w["bl1"] = np.asarray(inputs["bl1"], dtype=np.float32).reshape(64, 1).copy()
    w["wl2"] = np.ascontiguousarray(np.asarray(inputs["Wl2"], dtype=np.float32))
    w["bl2"] = np.asarray(inputs["bl2"], dtype=np.float32).reshape(2, 1).copy()
    return w


_IDENT = np.eye(128, dtype=np.float32)
_IDENTH = np.eye(128, dtype=np.float16)

_CACHED = {}


def _get_module():
    if "nc" in _CACHED:
        return _CACHED["nc"]
    nc = bacc.Bacc("TRN2", target_bir_lowering=False, debug=False, num_devices=N_CORES)
    ins = {"pos": nc.dram_tensor("pos", (G, 2, N), F32, kind="ExternalInput"),
           "ident": nc.dram_tensor("ident", (128, 128), F32, kind="ExternalInput"),
           "identh": nc.dram_tensor("identh", (128, 128), F16, kind="ExternalInput")}
    for li, (D, C) in enumerate(LAYERS):
        ins[f"wab{li}"] = nc.dram_tensor(f"wab{li}", (D, C), F32, kind="ExternalInput")
        ins[f"wb{li}"] = nc.dram_tensor(f"wb{li}", (D, C), F32, kind="ExternalInput")
        if li > 0:
            ins[f"wa{li}"] = nc.dram_tensor(f"wa{li}", (D, C), F32,
                                            kind="ExternalInput")
        ins[f"b{li}"] = nc.dram_tensor(f"b{li}", (C, 1), F32, kind="ExternalInput")
    ins["wl1"] = nc.dram_tensor("wl1", (2, 128, 64), F32, kind="ExternalInput")
    ins["bl1"] = nc.dram_tensor("bl1", (64, 1), F32, kind="ExternalInput")
    ins["wl2"] = nc.dram_tensor("wl2", (64, 2), F32, kind="ExternalInput")
    ins["bl2"] = nc.dram_tensor("bl2", (2, 1), F32, kind="ExternalInput")
    outs = {"out": nc.dram_tensor("out", (2, G), F32, kind="ExternalOutput")}
    with tile.TileContext(nc) as tc:
        _build(tc, nc, ins, outs, n_cores=N_CORES)
    nc.compile()
    _CACHED["nc"] = nc
    return nc


def kernel(**inputs):
    pos = np.ascontiguousarray(np.asarray(inputs["pos"], dtype=np.float32))
    pos16 = np.ascontiguousarray(
        pos.reshape(B_TOTAL, N, 2).transpose(0, 2, 1))
    w = _host_weights(inputs)
    nc = _get_module()
    in_maps = []
    for core in range(N_CORES):
        m = {"pos": np.ascontiguousarray(pos16[core * G:(core + 1) * G]),
             "ident": _IDENT, "identh": _IDENTH}
        m.update(w)
        in_maps.append(m)
    res = run_bass_kernel_spmd(nc, in_maps, list(range(N_CORES)))
    outs = [res.results[c]["out"].T for c in range(N_CORES)]  # each [G, 2]
    return np.concatenate(outs, axis=0).astype(np.float32)


# revision 27
# speedup vs baseline: 1.1654x; 1.1654x over previous
# Trainium2 Bass kernel for nn_Edge_CNN (DynamicEdgeConv x3 + PairNorm + pool + MLP head).
#
# Strategy: data-parallel over the 32 graphs -> 8 NeuronCores x 4 graphs. PairNorm
# couples all graphs (stats over the whole batch), handled with a tiny per-layer
# AllReduce of per-channel sums + total square sum.
#
# Key structure:
#  - PairNorm is a monotone per-channel affine (scalar scale s>0, per-channel
#    shift). kNN ordering is affine-invariant, and u+max_j v commutes with the
#    affine, so each layer computes on UNNORMALIZED fp32 features z and the
#    normalization collapses into the final relu: z_next = relu(s*m + ccol),
#    ccol = b - s*W[:D]^T mu. All collective-consuming ops are emitted AFTER the
#    next layer's selection work, so the per-layer AllReduce latency hides
#    behind compute (engines execute their streams in order).
#  - per layer, emission is phased (sel x4 graphs, agg x4, consume, relu x4) so
#    the in-order engine streams pipeline across graphs and layers.
#  - top-k (k=10, self-inclusive) per row: gram chunk in PSUM (fp32 matmuls,
#    -|x_j|^2/2 as a rank-1 accumulate), ACT-copied to SBUF, then
#    max8/max_index/match_replace/max8/max_index; slots 0..9 of the 16-slot
#    gather layout are ranks 1..10. Index transposes run on the PE (u16 idx ->
#    fp32 cast -> transpose-matmul vs identity -> ACT cast back).
#  - aggregation: gpsimd ap_gather of v columns (fp32; L3 packs its two 128-ch
#    blocks as fp16 pairs in one u32 gather); segmented max over slots 0..9 via
#    DVE strided tensor_reduce (L1/L2, fp16 out) or packed-fp16 TT trees (L3,
#    2x mode); maxv is added into the u PSUM accumulation with an fp16
#    identity matmul (L1/L2) and relu reads m fp16.
#
# kernel(**inputs) takes the FULL unsharded inputs and returns the FULL [32, 2].

import numpy as np
from contextlib import ExitStack

import concourse.bass as bass
import concourse.bacc as bacc
import concourse.mybir as mybir
import concourse.tile as tile
from concourse.bass_utils import run_bass_kernel_spmd

N = 1024
B_TOTAL = 32
N_CORES = 8
G = B_TOTAL // N_CORES
F32 = mybir.dt.float32
F16 = mybir.dt.float16
U16 = mybir.dt.uint16
I16 = mybir.dt.int16
AF = mybir.ActivationFunctionType
ALU = mybir.AluOpType
AX = mybir.AxisListType
NCHUNK = N // 128
LAYERS = [(2, 64), (64, 128), (128, 256)]  # (D_in, C_out)
HN = 128  # nodes per gather part


def _ccdiv(c):
    return (c + 127) // 128


def _build(tc, nc, ins, outs, n_cores, eps=1e-5):
    TOTAL_NODES = float(B_TOTAL * N)
    replica = [list(range(n_cores))]

    es = ExitStack()
    sb = es.enter_context(tc.tile_pool(name="sb", bufs=1))
    xp = es.enter_context(tc.tile_pool(name="xp", bufs=12))
    wk = es.enter_context(tc.tile_pool(name="wk", bufs=2))
    tcp = es.enter_context(tc.tile_pool(name="tcp", bufs=3))
    mp = es.enter_context(tc.tile_pool(name="mp", bufs=9))
    vt = es.enter_context(tc.tile_pool(name="vt", bufs=5))
    sm = es.enter_context(tc.tile_pool(name="sm", bufs=4))
    gt = es.enter_context(tc.tile_pool(name="gt", bufs=3))
    ndp = es.enter_context(tc.tile_pool(name="ndp", bufs=2))
    ps_g = es.enter_context(tc.tile_pool(name="ps_g", bufs=3, space="PSUM"))
    ps_t = es.enter_context(tc.tile_pool(name="ps_t", bufs=1, space="PSUM"))
    ps_v = es.enter_context(tc.tile_pool(name="ps_v", bufs=2, space="PSUM"))
    dr = es.enter_context(tc.tile_pool(name="dr", bufs=1, space="DRAM"))

    onesD = sb.tile([128, 1], F32, tag="onesD")
    nc.vector.memset(onesD[:], 1.0)
    ones1 = sb.tile([1, 128], F32, tag="ones1")
    nc.vector.memset(ones1[:], 1.0)
    ones128 = sb.tile([128, 1], F32, tag="ones128")
    nc.vector.memset(ones128[:], 1.0)
    rcol0 = sb.tile([128, 1], F32, tag="rcol0")
    nc.vector.memset(rcol0[:], 1.0)
    # z (layer-input feature) tiles (loaded first so layer 0 starts early)
    zT = {}
    for g in range(G):
        t = xp.tile([128, N], F32, tag="xh", name=f"x0_{g}")
        nc.sync.dma_start(t[0:2, :], ins["pos"][g, :, :])
        zT[(g, 0)] = t
    identF = sb.tile([128, 128], F32, tag="identF")
    nc.sync.dma_start(identF[:], ins["ident"][:])
    identH = sb.tile([128, 128], F16, tag="identH")
    nc.sync.dma_start(identH[:], ins["identh"][:])

    W = {}
    for li, (D, C) in enumerate(LAYERS):
        for nm in ("wab", "wb"):
            t = sb.tile([D, C], F32, tag=f"{nm}{li}", name=f"{nm}{li}")
            nc.sync.dma_start(t[:], ins[f"{nm}{li}"][:])
            W[f"{nm}{li}"] = t
        if li > 0:
            t = sb.tile([D, C], F32, tag=f"wa{li}", name=f"wa{li}")
            nc.sync.dma_start(t[:], ins[f"wa{li}"][:])
            W[f"wa{li}"] = t
        t = sb.tile([min(C, 128), _ccdiv(C)], F32, tag=f"b{li}", name=f"b{li}")
        nc.sync.dma_start(t[:], ins[f"b{li}"][:].rearrange("(cc p) one -> p (cc one)",
                                                           p=min(C, 128)))
        W[f"b{li}"] = t
    wl1 = sb.tile([128, 2, 64], F32, tag="wl1")
    nc.sync.dma_start(wl1[:], ins["wl1"][:].rearrange("cc p c -> p cc c"))
    wl2 = sb.tile([64, 2], F32, tag="wl2")
    nc.sync.dma_start(wl2[:], ins["wl2"][:])
    bl1 = sb.tile([64, 1], F32, tag="bl1")
    nc.sync.dma_start(bl1[:], ins["bl1"][:])
    bl2 = sb.tile([2, 1], F32, tag="bl2")
    nc.sync.dma_start(bl2[:], ins["bl2"][:])

    # per-layer normalization constants (written after each layer's collective)
    rcolL = {-1: rcol0}
    ccol = {}  # (li, cc) -> [128,1] f32 bias column for the relu of layer li
    for cc in range(_ccdiv(LAYERS[0][1])):
        ccol[(0, cc)] = None  # layer 0 uses b0 directly

    def unit_sel(li, D, C, g):
        CC = _ccdiv(C)
        zin = zT[(g, 0)]

        # ---- selection prep: rhsq = -0.5*sum_c z^2 (fp16 row)
        sqx = sm.tile([128, N], F32, tag="sqx")
        nc.scalar.activation(sqx[0:D, :], zin[0:D, :], AF.Square)
        psq = ps_v.tile([1, N], F32, tag="pvu")
        for b in range(2):
            sl = slice(512 * b, 512 * (b + 1))
            nc.tensor.matmul(psq[:, sl], onesD[0:D, :], sqx[0:D, sl],
                             start=True, stop=True)
        rhsq = sm.tile([1, N], F32, tag="rhsq")
        nc.scalar.activation(rhsq[:], psq[:], AF.Copy, scale=-0.5)

        # ---- per 128-row chunk: gram into PSUM, copy to SBUF (ACT), exact
        # top-16 selection on the SBUF copy; index transposes on the PE.
        wrapIdx = sm.tile([16, N], U16, tag="wrapIdx")
        for c in range(NCHUNK):
            nd = ndp.tile([128, N], F32, tag="nd")
            for b in range(2):
                sl = slice(512 * b, 512 * (b + 1))
                pg = ps_g.tile([128, 512], F32, tag="gram")
                nc.tensor.matmul(pg[:], zin[0:D, 128 * c:128 * (c + 1)],
                                 zin[0:D, sl], start=True, stop=False)
                nc.tensor.matmul(pg[:], ones1[:], rhsq[:, sl],
                                 start=False, stop=True)
                nc.scalar.activation(nd[:, sl], pg[:], AF.Copy)
            mx = sm.tile([128, 16], F32, tag="mx")
            half = (c % 2) * 16
            if half == 0:
                idx32 = sm.tile([128, 32], U16, tag="idx32")
            nc.vector.max(mx[:, 0:8], nd[:])
            nc.vector.max_index(idx32[:, half:half + 8], mx[:, 0:8], nd[:])
            nc.vector.match_replace(nd[:], mx[:, 0:8], nd[:], -1e30)
            nc.vector.max(mx[:, 8:16], nd[:])
            nc.vector.max_index(idx32[:, half + 8:half + 16], mx[:, 8:16], nd[:])
            if half == 16:
                idx32f = sm.tile([128, 32], F32, tag="idx32f")
                nc.scalar.activation(idx32f[:], idx32[:], AF.Copy)
                for h2 in range(2):
                    ptr = ps_t.tile([16, 128], F32, tag="ptr")
                    nc.tensor.transpose(ptr[:], idx32f[:, 16 * h2:16 * (h2 + 1)],
                                        identF[:])
                    nc.scalar.activation(
                        wrapIdx[0:16, 128 * (c - 1 + h2):128 * (c + h2)],
                        ptr[:], AF.Copy)

        repIdx = sm.tile([128, N], I16, tag="repIdx")
        for grp in range(8):
            nc.sync.dma_start(repIdx[16 * grp:16 * (grp + 1), :],
                              wrapIdx[0:16, :].bitcast(I16))

        # ---- v transforms (raw, unnormalized)
        if li < 2:
            vtt = vt.tile([128, N], F32, tag="vT")
            pv = ps_v.tile([128, N], F32, tag="pvu")
            for b in range(2):
                sl = slice(512 * b, 512 * (b + 1))
                nc.tensor.matmul(pv[0:C, sl], W[f"wb{li}"][0:D, :], zin[0:D, sl],
                                 start=True, stop=True)
            nc.scalar.activation(vtt[0:C, :], pv[0:C, :], AF.Copy)
        else:
            # pack the two 128-ch blocks as fp16 pairs into one fp32-typed tile
            vtt = vt.tile([128, N], F32, tag="vT")
            v16 = vtt[:].bitcast(F16).rearrange("p (n two) -> p n two", two=2)
            for cc in range(CC):
                csl = slice(128 * cc, 128 * (cc + 1))
                pv = ps_v.tile([128, N], F32, tag="pvu")
                for b in range(2):
                    sl = slice(512 * b, 512 * (b + 1))
                    nc.tensor.matmul(pv[:, sl], W[f"wb{li}"][0:D, csl], zin[0:D, sl],
                                     start=True, stop=True)
                nc.scalar.activation(v16[:, :, cc:cc + 1], pv[:], AF.Copy)

        return (repIdx, vtt)

    def unit_agg(li, D, C, g, sel):
        CC = _ccdiv(C)
        repIdx, vtt = sel
        zin = zT[(g, 0)]
        ch = min(C, 128)

        # ---- gather 16 slots per node (slots 0..9 = ranks 1..10), then
        # segmented max over slots 0..9 on DVE: strided tensor_reduce for
        # L1/L2 (fp32 -> fp16 maxv), packed-fp16-pair TT tree for L3 (2x mode).
        tC = tcp.tile([128, N], F32, tag="tC")
        tC16 = tC[:].bitcast(F16)
        for part in range(N // HN):
            hsl = slice(part * HN, (part + 1) * HN)
            gout = gt.tile([128, 16 * HN], F32, tag="gout", name=f"gout{part}")
            nc.gpsimd.ap_gather(gout[0:ch, :], vtt[0:ch, :], repIdx[0:ch, hsl],
                                channels=ch, num_elems=N, d=1, num_idxs=16 * HN)
            if li < 2:
                gv = gout[0:ch, :].rearrange("p (n s) -> p n s", s=16)
                nc.vector.tensor_reduce(tC16[0:ch, hsl], gv[:, :, 0:10],
                                        AX.X, ALU.max)
            else:
                gv = gout[:].bitcast(F16).rearrange("p (n s t) -> p n s t", s=16, t=2)
                tA = wk.tile([128, 5 * HN], F32, tag="tA")
                tAv = tA[:].bitcast(F16).rearrange("p (n s t) -> p n s t", s=5, t=2)
                nc.vector.tensor_tensor(tAv, gv[:, :, 0:5, :], gv[:, :, 5:10, :],
                                        ALU.max)
                tB = wk.tile([128, 2 * HN], F32, tag="tB")
                tBv = tB[:].bitcast(F16).rearrange("p (n s t) -> p n s t", s=2, t=2)
                nc.vector.tensor_tensor(tBv, tAv[:, :, 0:2, :], tAv[:, :, 2:4, :],
                                        ALU.max)
                tCv = tC16[:, 2 * part * HN:2 * (part + 1) * HN].rearrange(
                    "p (n one t) -> p n one t", one=1, t=2)
                nc.vector.tensor_tensor(tCv, tBv[:, :, 0:1, :], tBv[:, :, 1:2, :],
                                        ALU.max)
                nc.vector.tensor_tensor(tCv, tCv, tAv[:, :, 4:5, :], ALU.max)

        # ---- m = u + maxv (relu deferred to phase R). L1/L2: maxv folded into
        # the u PSUM accumulation via an fp16 identity matmul; L3: DVE adds the
        # strided fp16 maxv lane to the u PSUM directly.
        ms = {}
        for cc in range(CC):
            cw = min(128, C - 128 * cc)
            csl = slice(128 * cc, 128 * cc + cw)
            pu = ps_v.tile([128, N], F32, tag="pvu")
            for b in range(2):
                sl = slice(512 * b, 512 * (b + 1))
                nc.tensor.matmul(pu[0:cw, sl], W[f"wab{li}"][0:D, csl], zin[0:D, sl],
                                 start=True, stop=False)
                if li < 2:
                    nc.tensor.matmul(pu[0:cw, sl], identH[0:cw, 0:cw], tC16[0:cw, sl],
                                     start=False, stop=True)
                else:
                    nc.tensor.matmul(
                        pu[0:cw, sl], identH[0:cw, 0:cw],
                        tC16[:].rearrange("p (n t) -> p n t", t=2)[:, sl, cc:cc + 1],
                        start=False, stop=True)
            m = mp.tile([128, N], F16, tag="mT", name=f"m{li}_{g}_{cc}")
            nc.scalar.activation(m[0:cw, :], pu[0:cw, :], AF.Copy)
            ms[cc] = (m, cw)
        return ms

    def unit_relu(li, C, g, ms):
        hs = {}
        for cc, (m, cw) in ms.items():
            z = xp.tile([128, N], F32, tag="xh", name=f"h{li}_{g}_{cc}")
            rsum = sm.tile([128, 1], F32, tag="rsum")
            bias_col = W[f"b{li}"][0:cw, cc:cc + 1] if ccol[(li, cc)] is None \
                else ccol[(li, cc)][0:cw, :]
            nc.scalar.activation(z[0:cw, :], m[0:cw, :], AF.Relu,
                                 bias=bias_col, scale=rcolL[li - 1][0:cw, :],
                                 accum_out=rsum[0:cw, :])
            sqz = sm.tile([128, N], F16, tag="sqz")
            qsum = sm.tile([128, 1], F32, tag="qsum")
            nc.scalar.activation(sqz[0:cw, :], z[0:cw, :], AF.Square,
                                 accum_out=qsum[0:cw, :])
            hs[cc] = (z, rsum, qsum, cw)
        return hs

    def make_consume(li, C, CC, statg):
        def consume():
            # all collective-consuming ops; emitted late so the in-order
            # engine streams never stall on the collective latency.
            statv = sm.tile([128, 2 * CC], F32, tag=f"statv{li}", name=f"statv{li}")
            nc.vector.tensor_reduce(
                statv[:], statg[:].rearrange("p k c -> p c k"), AX.X, ALU.add)
            mu = sm.tile([128, CC], F32, tag=f"mu{li}", name=f"mu{li}")
            musq = sm.tile([128, CC], F32, tag=f"musq{li}", name=f"musq{li}")
            for cc in range(CC):
                nc.vector.tensor_scalar_mul(mu[:, cc:cc + 1],
                                            statv[:, 2 * cc:2 * cc + 1],
                                            1.0 / TOTAL_NODES)
            nc.vector.tensor_tensor(musq[:], mu[:], mu[:], ALU.mult)
            pr = ps_v.tile([1, 2], F32, tag="pvu")
            for cc in range(CC):
                nc.tensor.matmul(pr[:, 0:1], statv[:, 2 * cc + 1:2 * cc + 2],
                                 ones128[:], start=(cc == 0), stop=(cc == CC - 1))
            for cc in range(CC):
                nc.tensor.matmul(pr[:, 1:2], musq[:, cc:cc + 1], ones128[:],
                                 start=(cc == 0), stop=(cc == CC - 1))
            sc = sm.tile([1, 2], F32, tag=f"sc{li}", name=f"sc{li}")
            nc.scalar.activation(sc[:], pr[:], AF.Copy)
            rsc = sm.tile([1, 1], F32, tag=f"rsc{li}", name=f"rsc{li}")
            nc.vector.tensor_scalar(rsc[:], sc[:, 0:1], scalar1=1.0 / TOTAL_NODES,
                                    scalar2=eps, op0=ALU.mult, op1=ALU.add)
            nc.vector.tensor_tensor(rsc[:], rsc[:], sc[:, 1:2], ALU.subtract)
            nc.vector.reciprocal(rsc[:], rsc[:])
            nc.scalar.activation(rsc[:], rsc[:], AF.Sqrt)
            prb = ps_v.tile([128, 1], F32, tag="pvu")
            nc.tensor.matmul(prb[:], ones1[:], rsc[:], start=True, stop=True)
            rcol = sm.tile([128, 1], F32, tag=f"rcol{li}", name=f"rcol{li}")
            nc.scalar.activation(rcol[:], prb[:], AF.Copy)
            rcolL[li] = rcol
            if li < 2:
                nrcol = sm.tile([128, 1], F32, tag=f"nrcol{li}", name=f"nrcol{li}")
                nc.vector.tensor_scalar_mul(nrcol[:], rcol[:], -1.0)
                D2, C2 = LAYERS[li + 1]
                for cc2 in range(_ccdiv(C2)):
                    cw2 = min(128, C2 - 128 * cc2)
                    csl2 = slice(128 * cc2, 128 * cc2 + cw2)
                    pc = ps_v.tile([128, 1], F32, tag="pvu")
                    nc.tensor.matmul(pc[0:cw2, :], W[f"wa{li + 1}"][0:D2, csl2],
                                     mu[0:D2, 0:1], start=True, stop=True)
                    cl = sm.tile([128, 1], F32, tag=f"ccol{li + 1}_{cc2}",
                                 name=f"ccol{li + 1}_{cc2}")
                    nc.scalar.activation(cl[0:cw2, :], pc[0:cw2, :], AF.Identity,
                                         scale=nrcol[0:cw2, :],
                                         bias=W[f"b{li + 1}"][0:cw2, cc2:cc2 + 1])
                    ccol[(li + 1, cc2)] = cl
            else:
                muR = sm.tile([128, CC], F32, tag=f"muR{li}", name=f"muR{li}")
                nc.vector.tensor_scalar(muR[:], mu[:], scalar1=rcol[:], scalar2=None,
                                        op0=ALU.mult)
                ccol["muR3"] = muR
        return consume

    pending = None
    hsg = {}
    for li, (D, C) in enumerate(LAYERS):
        CC = _ccdiv(C)
        sels = {}
        for g in range(G):
            sels[g] = unit_sel(li, D, C, g)
        msg_ = {}
        for g in range(G):
            msg_[g] = unit_agg(li, D, C, g, sels[g])
        if pending is not None:
            pending()
        ssum = [sm.tile([128, G], F32, tag=f"ssum{li}_{cc}", name=f"ssum{li}_{cc}")
                for cc in range(CC)]
        qsum = [sm.tile([128, G], F32, tag=f"qsum{li}_{cc}", name=f"qsums{li}_{cc}")
                for cc in range(CC)]
        hsg = {}
        for g in range(G):
            hs = unit_relu(li, C, g, msg_[g])
            for cc, (h, rs, qs, cw) in hs.items():
                nc.vector.tensor_copy(ssum[cc][0:cw, g:g + 1], rs[0:cw, :])
                nc.vector.tensor_copy(qsum[cc][0:cw, g:g + 1], qs[0:cw, :])
                hsg[(g, cc)] = (h, cw)
        stat = sm.tile([128, 2 * CC], F32, tag=f"stat{li}", name=f"stat{li}")
        nc.vector.memset(stat[:], 0.0)
        for cc in range(CC):
            cw = min(128, C - 128 * cc)
            nc.vector.tensor_reduce(stat[0:cw, 2 * cc:2 * cc + 1], ssum[cc][0:cw, :],
                                    AX.X, ALU.add)
            nc.vector.tensor_reduce(stat[0:cw, 2 * cc + 1:2 * cc + 2], qsum[cc][0:cw, :],
                                    AX.X, ALU.add)
        bi = dr.tile([128, 2 * CC], F32, tag=f"cc_in{li}", name=f"cc_in{li}")
        bo = dr.tile([8, 128, 2 * CC], F32, tag=f"cc_out{li}", name=f"cc_out{li}")
        nc.sync.dma_start(bi[:], stat[:])
        nc.gpsimd.collective_compute("AllGather", ALU.bypass, replica_groups=replica,
                                     ins=[bi[:].opt()], outs=[bo[:].opt()])
        statg = sm.tile([128, 8, 2 * CC], F32, tag=f"statg{li}", name=f"statg{li}")
        nc.sync.dma_start(statg[:], bo[:].rearrange("k p c -> p k c"))
        pending = make_consume(li, C, CC, statg)
        if li < 2:
            for g in range(G):
                zT[(g, 0)] = hsg[(g, 0)][0]

    # ---- global max pool (on raw z3, before the last collective lands) +
    # affine + MLP head
    gmat = sm.tile([128, 2, G], F32, tag="gmat")
    for g in range(G):
        for cc in range(2):
            h, cw = hsg[(g, cc)]
            nc.vector.tensor_reduce(gmat[:, cc, g:g + 1], h[:], AX.X, ALU.max)
    pending()
    muR = ccol["muR3"]
    rcol3 = rcolL[2]
    for cc in range(2):
        nc.vector.tensor_scalar(gmat[:, cc, :], gmat[:, cc, :], scalar1=rcol3[:],
                                scalar2=muR[:, cc:cc + 1], op0=ALU.mult,
                                op1=ALU.subtract)
    ph = ps_v.tile([64, G], F32, tag="pvu")
    for cc in range(2):
        nc.tensor.matmul(ph[:], wl1[:, cc, :], gmat[:, cc, :], start=(cc == 0),
                         stop=(cc == 1))
    hh = sm.tile([64, G], F32, tag="hh")
    nc.scalar.activation(hh[:], ph[:], AF.Relu, bias=bl1[:])
    po = ps_v.tile([2, G], F32, tag="pvu")
    nc.tensor.matmul(po[:], wl2[:], hh[:], start=True, stop=True)
    oo = sm.tile([2, G], F32, tag="oo")
    nc.scalar.activation(oo[:], po[:], AF.Identity, bias=bl2[:])
    nc.sync.dma_start(outs["out"][:], oo[:])
    es.close()


def _host_weights(inputs):
    w = {}
    for li, (D, C) in enumerate(LAYERS):
        Wl = np.asarray(inputs[f"W{li + 1}"], dtype=np.float32)
        w[f"wab{li}"] = np.ascontiguousarray(Wl[:D] - Wl[D:])
        w[f"wb{li}"] = np.ascontiguousarray(Wl[D:])
        if li > 0:
            w[f"wa{li}"] = np.ascontiguousarray(Wl[:D])
        w[f"b{li}"] = np.ascontiguousarray(
            np.asarray(inputs[f"b{li + 1}"], dtype=np.float32).reshape(C, 1))
    w["wl1"] = np.ascontiguousarray(
        np.asarray(inputs["Wl1"], dtype=np.float32).reshape(2, 128, 64))
    w["bl1"] = np.asarray(inputs["bl1"], dtype=np.float32).reshape(64, 1).copy()
    w["wl2"] = np.ascontiguousarray(np.asarray(inputs["Wl2"], dtype=np.float32))
    w["bl2"] = np.asarray(inputs["bl2"], dtype=np.float32).reshape(2, 1).copy()
    return w


_IDENT = np.eye(128, dtype=np.float32)
_IDENTH = np.eye(128, dtype=np.float16)

_CACHED = {}


def _get_module():
    if "nc" in _CACHED:
        return _CACHED["nc"]
    nc = bacc.Bacc("TRN2", target_bir_lowering=False, debug=False, num_devices=N_CORES)
    ins = {"pos": nc.dram_tensor("pos", (G, 2, N), F32, kind="ExternalInput"),
           "ident": nc.dram_tensor("ident", (128, 128), F32, kind="ExternalInput"),
           "identh": nc.dram_tensor("identh", (128, 128), F16, kind="ExternalInput")}
    for li, (D, C) in enumerate(LAYERS):
        ins[f"wab{li}"] = nc.dram_tensor(f"wab{li}", (D, C), F32, kind="ExternalInput")
        ins[f"wb{li}"] = nc.dram_tensor(f"wb{li}", (D, C), F32, kind="ExternalInput")
        if li > 0:
            ins[f"wa{li}"] = nc.dram_tensor(f"wa{li}", (D, C), F32,
                                            kind="ExternalInput")
        ins[f"b{li}"] = nc.dram_tensor(f"b{li}", (C, 1), F32, kind="ExternalInput")
    ins["wl1"] = nc.dram_tensor("wl1", (2, 128, 64), F32, kind="ExternalInput")
    ins["bl1"] = nc.dram_tensor("bl1", (64, 1), F32, kind="ExternalInput")
    ins["wl2"] = nc.dram_tensor("wl2", (64, 2), F32, kind="ExternalInput")
    ins["bl2"] = nc.dram_tensor("bl2", (2, 1), F32, kind="ExternalInput")
    outs = {"out": nc.dram_tensor("out", (2, G), F32, kind="ExternalOutput")}
    with tile.TileContext(nc) as tc:
        _build(tc, nc, ins, outs, n_cores=N_CORES)
    nc.compile()
    _CACHED["nc"] = nc
    return nc


def kernel(**inputs):
    pos = np.ascontiguousarray(np.asarray(inputs["pos"], dtype=np.float32))
    pos16 = np.ascontiguousarray(
        pos.reshape(B_TOTAL, N, 2).transpose(0, 2, 1))
    w = _host_weights(inputs)
    nc = _get_module()
    in_maps = []
    for core in range(N_CORES):
        m = {"pos": np.ascontiguousarray(pos16[core * G:(core + 1) * G]),
             "ident": _IDENT, "identh": _IDENTH}
        m.update(w)
        in_maps.append(m)
    res = run_bass_kernel_spmd(nc, in_maps, list(range(N_CORES)))
    outs = [res.results[c]["out"].T for c in range(N_CORES)]  # each [G, 2]
    return np.concatenate(outs, axis=0).astype(np.float32)
